# revision 10
# baseline (speedup 1.0000x reference)
"""Trainium2 Bass kernel for nn_DarcyResidual (P=256, B=128, 8 NeuronCores).

Math (reference):
    a = (x0 + 1.5) / 0.2,  p = (x1 + 0.9) / 115
    residual = -a*(p_d00 + p_d11) - a_d0*p_d0 - a_d1*p_d1 - 1
2nd-order central differences inside, 2nd-order one-sided at borders,
h = 1/256 on both axes.

Folded form computed here (G = 5/(460 h^2) = 65536/92):
    residual = -G * [ X0'*U4 + S1*R1 + C1a*C1p ] - 1
      X0' = X0 + 1.5  (added on host; all stencil row sums are 0 so the
                       shift does not change any derivative)
      U4  = 4*(rowD2raw(X1) + colD2raw(X1))   (raw h^2-scaled 2nd diffs)
      R1  = rowD1raw(X1), S1 = rowD1raw(X0')  (raw 2h-scaled 1st diffs)
      C1p = colD1raw(X1), C1a = colD1raw(X0')

v3 layout per core (16 images): SBUF [partition = row-within-128-block,
free = (row-block k:2, image b:2, col j:256)], 8 chunks of 2 images.

Engine assignment per chunk:
  PE:   R1 = D1@X1 and S1 = D1@X0' as single fp8 DoubleRow matmuls
        (diag + cross-block k-planes fused, 2x fp8 throughput; fp8
        input error only perturbs the minor grad-product term);
        U4 row part as bf16 WR2 diag matmuls + contract-1 cross fixups;
        U4 col part as 4I matmuls on +-1-shifted bf16 rhs views;
        final merge res = I@tm + I@(t2+t3) into PSUM.
  ACT:  evacuate S1|R1 PSUM -> bf16 (shift-1), final fused
        evacuate+affine(-G,-1), edge-column writes.
  DVE:  C1p/C1a shifted subs (2x), tm = stt(X0'*U4psum) merged [1022],
        t2 = TT(S1b*R1b) (2x), sum1 = t2+t3 (2x).
  GP:   t3 = C1a*C1p, output DMAs (SWDGE).

Border columns j=0,255 keep the f32r edge pipeline fed by a
host-pregathered edge tensor.  Output is bf16 (upcast on host).
"""

import numpy as np

P = 256
B = 128
NCORES = 8
BPC = B // NCORES          # images per core = 16
CHUNKS = 8
BCH = BPC // CHUNKS        # images per chunk = 2
FCH = 2 * BCH * P          # chunk free size = 1024
GAMMA = 5.0 * 65536.0 / 460.0

_cache = {}


def _stencils():
    D1 = np.zeros((P, P), dtype=np.float64)
    for i in range(1, P - 1):
        D1[i, i - 1] = -1.0
        D1[i, i + 1] = 1.0
    D1[0, 0:3] = [-3.0, 4.0, -1.0]
    D1[P - 1, P - 3:P] = [1.0, -4.0, 3.0]

    D2 = np.zeros((P, P), dtype=np.float64)
    for i in range(1, P - 1):
        D2[i, i - 1] = 1.0
        D2[i, i] = -2.0
        D2[i, i + 1] = 1.0
    D2[0, 0:4] = [2.0, -5.0, 4.0, -1.0]
    D2[P - 1, P - 4:P] = [-1.0, 4.0, -5.0, 2.0]
    return D1, D2


def _weights_bf16():
    """[128, 14, 128] bf16 lhsT blocks (same layout as v2):
    0-3: D1 blocks; 4-7: 4*(D2-2I) blocks; 8: 4I; 9: I;
    10-13: 4*D2 blocks (edge pipeline).
    All entries are small integers -> exact in bf16."""
    import ml_dtypes
    D1, D2 = _stencils()
    WR2 = 4.0 * (D2 - 2.0 * np.eye(P))
    WR2E = 4.0 * D2
    wtb = np.zeros((128, 14, 128), dtype=np.float64)
    for m in range(2):
        for kb in range(2):
            blk = lambda W: W[m * 128:(m + 1) * 128, kb * 128:(kb + 1) * 128].T
            wtb[:, m * 2 + kb, :] = blk(D1)
            wtb[:, 4 + m * 2 + kb, :] = blk(WR2)
            wtb[:, 10 + m * 2 + kb, :] = blk(WR2E)
    wtb[:, 8, :] = 4.0 * np.eye(128)
    wtb[:, 9, :] = np.eye(128)
    return wtb.astype(ml_dtypes.bfloat16)


def _weights_fp8():
    """[128, 2, 2, 128] fp8e4 DoubleRow lhsT: block m holds the two
    k-plane D1 blocks (diag + cross) for output row-block m.
    D1 entries are small integers -> exact in e4m3."""
    import ml_dtypes
    D1, _ = _stencils()
    w = np.zeros((128, 2, 2, 128), dtype=np.float64)
    for m in range(2):
        for kb in range(2):
            w[:, m, kb, :] = D1[m * 128:(m + 1) * 128,
                                kb * 128:(kb + 1) * 128].T
    return w.astype(ml_dtypes.float8_e4m3)


def _build_program():
    from concourse import bacc
    import concourse.mybir as mybir
    from concourse.tile import TileContext

    f32 = mybir.dt.float32
    bf16 = mybir.dt.bfloat16
    f8 = mybir.dt.float8e4
    ADD = mybir.AluOpType.add
    SUB = mybir.AluOpType.subtract
    MUL = mybir.AluOpType.mult
    COPY = mybir.ActivationFunctionType.Copy
    DR = mybir.MatmulPerfMode.DoubleRow

    nc = bacc.Bacc("TRN2", target_bir_lowering=False, debug=False,
                   num_devices=NCORES)
    xe = nc.dram_tensor("xe", [128, 2, 2, BPC, 8], bf16, kind="ExternalInput")
    xb0 = nc.dram_tensor("xb0", [CHUNKS, 128, 2, BCH, P], bf16,
                         kind="ExternalInput")
    xb1 = nc.dram_tensor("xb1", [CHUNKS, 128, 2, BCH, P], bf16,
                         kind="ExternalInput")
    xf0 = nc.dram_tensor("xf0", [CHUNKS, 128, 2, BCH, P], f8,
                         kind="ExternalInput")
    xf1 = nc.dram_tensor("xf1", [CHUNKS, 128, 2, BCH, P], f8,
                         kind="ExternalInput")
    wtbd = nc.dram_tensor("wtbd", [128, 14, 128], bf16, kind="ExternalInput")
    wdrd = nc.dram_tensor("wdrd", [128, 2, 2, 128], f8, kind="ExternalInput")
    # X1 row 127 per chunk, relocated to partition 0 (PE rhs base must be
    # 0/32/64; row 127 lives at partition 127 in the main layout)
    xr1d = nc.dram_tensor("xr1d", [1, CHUNKS, BCH, P], bf16,
                          kind="ExternalInput")
    yout = nc.dram_tensor("yout", [CHUNKS, 128, 2, BCH, P], bf16,
                          kind="ExternalOutput")

    with TileContext(nc) as tc:
        with (
            tc.tile_pool(name="const", bufs=1) as cpool,
            tc.tile_pool(name="edge", bufs=1) as epool,
            tc.tile_pool(name="work", bufs=2) as pool,
            tc.tile_pool(name="psum", bufs=2, space="PSUM") as pp,
        ):
            wtb = cpool.tile([128, 14, 128], bf16)
            nc.sync.dma_start(out=wtb[:], in_=wtbd[:])
            wdr = cpool.tile([128, 2, 2, 128], f8)
            nc.sync.dma_start(out=wdr[:], in_=wdrd[:])
            xr1 = cpool.tile([1, CHUNKS, BCH, P], bf16)
            nc.sync.dma_start(out=xr1[:], in_=xr1d[:])

            def Wb(i):
                return wtb[:, i, :]

            stt = nc.vector.scalar_tensor_tensor

            def emit_edge():
                # ------------- edge pipeline (output cols j=0 and j=255) ----
                X0e = epool.tile([128, 2, BPC, 8], bf16)
                X1e = epool.tile([128, 2, BPC, 8], bf16)
                nc.sync.dma_start(out=X0e[:], in_=xe[:, 0])
                nc.sync.dma_start(out=X1e[:], in_=xe[:, 1])

                X0ef = X0e.rearrange("p k b c -> p (k b c)")
                X1ef = X1e.rearrange("p k b c -> p (k b c)")
                # [128, 32, 8] views
                E1 = X1e.rearrange("p k b c -> p (k b) c")
                E0 = X0e.rearrange("p k b c -> p (k b) c")

                def et(name, d=2):
                    return epool.tile([128, 2 * BPC, d], f32, name=name,
                                      tag=name)

                # all three edge stencil outputs packed into one "sr"-tag
                # PSUM slot (768 f32 <= 1024 f32 slot)
                esr = pp.tile([128, 3, 2, BPC, 8], f32, tag="sr")
                S1e, R1e, R2e = esr[:, 0], esr[:, 1], esr[:, 2]
                R2ef = R2e.rearrange("p k b c -> p (k b c)")
                R1ef = R1e.rearrange("p k b c -> p (k b c)")
                S1ef = S1e.rearrange("p k b c -> p (k b c)")
                for m in range(2):
                    osl = slice(m * 128, (m + 1) * 128)
                    for kb in range(2):
                        isl = slice(kb * 128, (kb + 1) * 128)
                        st, sp = kb == 0, kb == 1
                        nc.tensor.matmul(R1ef[:, osl], Wb(m * 2 + kb),
                                         X1ef[:, isl], start=st, stop=sp)
                        nc.tensor.matmul(S1ef[:, osl], Wb(m * 2 + kb),
                                         X0ef[:, isl], start=st, stop=sp)
                        nc.tensor.matmul(R2ef[:, osl], Wb(10 + m * 2 + kb),
                                         X1ef[:, isl], start=st, stop=sp)

                # paired forward/mirrored diffs: half 0 = j=0 side (fwd),
                # half 1 = j=255 side (also forward-oriented: f7-f6 etc.)
                a1, b1, c1 = et("a1"), et("b1"), et("c1")
                a0, b0 = et("a0"), et("b0")
                nc.vector.tensor_sub(a1[:], E1[:, :, 1:8:6], E1[:, :, 0:7:6])
                nc.vector.tensor_sub(b1[:], E1[:, :, 2:7:4], E1[:, :, 1:6:4])
                nc.vector.tensor_sub(c1[:], E1[:, :, 3:6:2], E1[:, :, 2:5:2])
                nc.vector.tensor_sub(a0[:], E0[:, :, 1:8:6], E0[:, :, 0:7:6])
                nc.vector.tensor_sub(b0[:], E0[:, :, 2:7:4], E0[:, :, 1:6:4])

                # one-sided raw stencils (Z sign flips on the mirror half)
                q, Z = et("q"), et("Z")
                C1pe, C1ae = et("C1pe"), et("C1ae")
                stt(q[:], b1[:], 3.0, c1[:], MUL, SUB)      # 3b - c
                stt(Z[:], a1[:], -2.0, q[:], MUL, ADD)      # -2a + 3b - c
                stt(C1pe[:], a1[:], 3.0, b1[:], MUL, SUB)   # 3a - b
                stt(C1ae[:], a0[:], 3.0, b0[:], MUL, SUB)

                RP2 = R2e.rearrange("p k b c -> p (k b) c")
                RP1 = R1e.rearrange("p k b c -> p (k b) c")
                U4e, tme, t2e = et("U4e"), et("tme"), et("t2e")
                stt(U4e[:, :, 0:1], Z[:, :, 0:1], 4.0, RP2[:, :, 0:1],
                    MUL, ADD)
                stt(U4e[:, :, 1:2], Z[:, :, 1:2], -4.0, RP2[:, :, 7:8],
                    MUL, ADD)

                Scpe = epool.tile([128, 2, BPC, 8], f32)
                nc.scalar.copy(out=Scpe.rearrange("p k b c -> p (k b c)"),
                               in_=S1ef[:])
                SP = Scpe.rearrange("p k b c -> p (k b) c")

                # X0 input is already X0+1.5 -> no additive shift here
                stt(tme[:], E0[:, :, 0:8:7], 0.0, U4e[:], ADD, MUL)
                nc.vector.tensor_mul(t2e[:], SP[:, :, 0:8:7], RP1[:, :, 0:8:7])
                nc.vector.tensor_add(tme[:], tme[:], t2e[:])
                nc.vector.tensor_mul(C1ae[:], C1ae[:], C1pe[:])  # t3e in-place
                nc.vector.tensor_add(tme[:], tme[:], C1ae[:])
                rese = epool.tile([128, 2, BPC, 2], f32)
                nc.scalar.activation(
                    rese.rearrange("p k b e -> p (k b) e"), tme[:], COPY,
                    bias=-1.0, scale=-GAMMA)

                return rese

            # ------------- main pipeline, 8 chunks of 2 images -------------
            for c in range(CHUNKS):
                # bf16 x1 into 2-left-padded tile (shifted 4I rhs views);
                # x0' plain; fp8 copies for the DoubleRow D1 matmuls
                Xp = pool.tile([128, FCH + 4], bf16, tag="x1", bufs=3)
                nc.sync.dma_start(
                    out=Xp[:, 2:FCH + 2].rearrange(
                        "p (k b j) -> p k b j", k=2, b=BCH),
                    in_=xb1[c])
                X0c = pool.tile([128, 2, BCH, P], bf16, tag="x0", bufs=3)
                nc.sync.dma_start(out=X0c[:], in_=xb0[c])
                F1 = pool.tile([128, 2, BCH * P], f8, tag="f1", bufs=3)
                nc.sync.dma_start(
                    out=F1.rearrange("p k (b j) -> p k b j", b=BCH),
                    in_=xf1[c])
                F0 = pool.tile([128, 2, BCH * P], f8, tag="f0", bufs=3)
                nc.sync.dma_start(
                    out=F0.rearrange("p k (b j) -> p k b j", b=BCH),
                    in_=xf0[c])

                X0f = X0c.rearrange("p k b j -> p (k b j)")
                X1f = Xp[:, 2:FCH + 2]

                C1p = pool.tile([128, FCH], bf16, tag="c1p", bufs=3)
                C1a = pool.tile([128, FCH], bf16, tag="c1a", bufs=3)
                t3b = pool.tile([128, FCH], bf16, tag="t3b", bufs=3)
                t2b = pool.tile([128, FCH], bf16, tag="t2b", bufs=3)
                sm1 = pool.tile([128, FCH], bf16, tag="sm1", bufs=3)
                tm = pool.tile([128, FCH], bf16, tag="tm", bufs=3)
                SRb = pool.tile([128, 2, 2, P * BCH - 1], bf16, tag="srb",
                                bufs=3)

                # column stencils, shifted layout (slot t = col t+1),
                # all views 4-byte aligned -> 2x mode
                nc.vector.tensor_sub(C1p[:, 0:FCH - 2], Xp[:, 4:FCH + 2],
                                     Xp[:, 2:FCH])
                nc.vector.tensor_sub(C1a[:, 0:FCH - 2], X0f[:, 2:FCH],
                                     X0f[:, 0:FCH - 2])
                # t3 on GPSIMD (SBUF-only op; frees DVE)
                nc.gpsimd.tensor_mul(t3b[:], C1a[:], C1p[:])

                # S1 | R1: one fp8 DoubleRow matmul each per m (diag+cross
                # k-planes fused).  sr tile [128, 2, 512]: plane 0 = S1,
                # plane 1 = R1 for row-block m.
                u4 = pp.tile([128, 2, BCH * P], f32, name=f"u4_{c}",
                             tag="u4", bufs=1)
                for m in range(2):
                    sr = pp.tile([128, 2, BCH * P], f32, name=f"sr_{c}_{m}",
                                 tag="sr", bufs=2)
                    nc.tensor.matmul(sr[:, 0, :], wdr[:, m], F0[:],
                                     start=True, stop=True, perf_mode=DR)
                    nc.tensor.matmul(sr[:, 1, :], wdr[:, m], F1[:],
                                     start=True, stop=True, perf_mode=DR)

                    # U4 row part: WR2 diag block + contract-1 cross fixup
                    isl = slice(m * (BCH * P), (m + 1) * (BCH * P))
                    nc.tensor.matmul(u4[:, m, :], Wb(4 + m * 3), X1f[:, isl],
                                     start=True, stop=False)
                    if m == 0:
                        # out row 127 += 4 * X1[row 128]
                        nc.tensor.matmul(u4[:, 0, :], wtb[0:1, 5, :],
                                         Xp[0:1, 2 + BCH * P:2 + FCH],
                                         start=False, stop=False)
                    else:
                        # out row 128 += 4 * X1[row 127]; lhsT 4*e0 is
                        # row 0 of the 4I block
                        nc.tensor.matmul(
                            u4[:, 1, :], wtb[0:1, 8, :],
                            xr1[:, c].rearrange("p b j -> p (b j)"),
                            start=False, stop=False)
                    # U4 col part: 4I on +-1-shifted rhs views
                    lo = m * (BCH * P)
                    hi = lo + BCH * P
                    nc.tensor.matmul(u4[:, m, :], Wb(8),
                                     Xp[:, lo + 3:hi + 3],
                                     start=False, stop=False)
                    nc.tensor.matmul(u4[:, m, :], Wb(8),
                                     Xp[:, lo + 1:hi + 1],
                                     start=False, stop=True)

                    # ACT evacuates S1|R1 with the shift-1 baked in
                    nc.scalar.copy(out=SRb[:, m], in_=sr[:, :, 1:BCH * P])

                # tm = X0' * U4, single merged stt straight from PSUM
                u4f = u4.rearrange("p m f -> p (m f)")
                stt(tm[:, 0:FCH - 2], X0f[:, 1:FCH - 1], 1.0,
                    u4f[:, 1:FCH - 1], MUL, MUL)

                # t2 = S1*R1 (bf16 2x), sum1 = t2 + t3 (bf16 2x)
                t2v = t2b.rearrange("p (m u) -> p m u", m=2)[:, :, 0:BCH * P - 1]
                nc.vector.tensor_mul(t2v, SRb[:, :, 0, :], SRb[:, :, 1, :])
                nc.vector.tensor_add(sm1[:], t2b[:], t3b[:])

                # final merge on PE: res = I@tm + I@sum1 per 512-half
                res = pp.tile([128, 2, BCH, P], f32, name=f"res_{c}",
                              tag="res", bufs=1)
                resf = res.rearrange("p k b j -> p (k b j)")
                H = FCH // 2
                nc.tensor.matmul(resf[:, 0:H - 1], Wb(9), tm[:, 0:H - 1],
                                 start=True, stop=False)
                nc.tensor.matmul(resf[:, 0:H - 1], Wb(9), sm1[:, 0:H - 1],
                                 start=False, stop=True)
                nc.tensor.matmul(resf[:, H:FCH - 1], Wb(9), tm[:, H:FCH - 1],
                                 start=True, stop=False)
                nc.tensor.matmul(resf[:, H:FCH - 1], Wb(9), sm1[:, H:FCH - 1],
                                 start=False, stop=True)

                if c == 0:
                    rese = emit_edge()
                outt = pool.tile([128, 2, BCH, P], bf16, tag="out", bufs=3)
                # res slot t = col t+1 -> out col j reads slot j-1
                nc.scalar.activation(outt[:, :, :, 1:P - 1],
                                     res[:, :, :, 0:P - 2], COPY,
                                     bias=-1.0, scale=-GAMMA)
                # edge columns j=0,255 from the edge pipeline (one copy)
                nc.scalar.copy(out=outt[:, :, :, 0:P:P - 1],
                               in_=rese[:, :, c * BCH:(c + 1) * BCH, :])
                nc.gpsimd.dma_start(out=yout[c], in_=outt[:])

    nc.compile()
    return nc


def _get_program():
    if "nc" not in _cache:
        _cache["nc"] = _build_program()
        _cache["wtbd"] = _weights_bf16()
        _cache["wdrd"] = _weights_fp8()
    return _cache["nc"], _cache["wtbd"], _cache["wdrd"]


def _shard_inputs(x0_pred):
    import ml_dtypes
    x = np.ascontiguousarray(np.asarray(x0_pred, dtype=np.float32))
    _, wtbd, wdrd = _get_program()
    in_maps = []
    for i in range(NCORES):
        shard = x[i * BPC:(i + 1) * BPC]                      # [16,2,256,256]
        x0p = shard[:, 0] + 1.5                               # [16,256,256]
        x1 = shard[:, 1]
        # [chunks, 128, k, b, j] layout: img = 2c+b, row = 128k+p

        def to_chunks(a):
            # [16,256,256] -> [8,2,2,128,256] (c,b,k,p,j) -> (c,p,k,b,j)
            r = a.reshape(CHUNKS, BCH, 2, 128, P).transpose(0, 3, 2, 1, 4)
            return np.ascontiguousarray(r)

        xb0 = to_chunks(x0p).astype(ml_dtypes.bfloat16)
        xb1 = to_chunks(x1).astype(ml_dtypes.bfloat16)
        xf0 = to_chunks(x0p).astype(ml_dtypes.float8_e4m3)
        xf1 = to_chunks(x1).astype(ml_dtypes.float8_e4m3)
        # X1 row 127: [16,256] -> [1, 8c, 2b, 256]
        xr1 = np.ascontiguousarray(
            x1[:, 127, :].reshape(1, CHUNKS, BCH, P)).astype(
                ml_dtypes.bfloat16)

        # edge tensor [128, ch, k, b16, 8cols] from x0', x1
        both = np.stack([x0p, x1], axis=1)                    # [16,2,256,256]
        arr = both.reshape(BPC, 2, 2, 128, P).transpose(3, 1, 2, 0, 4)
        cols = [0, 1, 2, 3, P - 4, P - 3, P - 2, P - 1]
        xe = np.ascontiguousarray(arr[:, :, :, :, cols]).astype(
            ml_dtypes.bfloat16)
        in_maps.append({"xe": xe, "xb0": xb0, "xb1": xb1, "xf0": xf0,
                        "xf1": xf1, "xr1d": xr1, "wtbd": wtbd, "wdrd": wdrd})
    return in_maps


def _unshard(results):
    outs = []
    for i in range(NCORES):
        y = np.asarray(results[i]["yout"], dtype=np.float32)
        # [8, 128, 2, 2, 256] (c,p,k,b,j) -> img 2c+b, row 128k+p
        y = y.transpose(0, 3, 2, 1, 4).reshape(BPC, 1, P, P)
        outs.append(y)
    return np.ascontiguousarray(np.concatenate(outs, axis=0))


def _run(x0_pred, trace=False, tmpdir=None):
    import time
    from concourse.bass_utils import run_bass_kernel_spmd
    nc = _get_program()[0]
    in_maps = _shard_inputs(x0_pred)
    try:
        res = run_bass_kernel_spmd(nc, in_maps, list(range(NCORES)),
                                   trace=trace, tmpdir=tmpdir)
    except Exception:
        # transient NRT execution failures have been observed; one retry
        time.sleep(2.0)
        res = run_bass_kernel_spmd(nc, in_maps, list(range(NCORES)),
                                   trace=trace, tmpdir=tmpdir)
    return _unshard(res.results), res


def kernel(x0_pred):
    out, _ = _run(x0_pred, trace=False)
    return out
